# revision 11
# baseline (speedup 1.0000x reference)
"""Causal self-attention (B=4, T=2048, C=1024, H=16) on 8 TRN2 NeuronCores.

Sharding: core c = (b, hg) with b = c//2 batch index, hg = c%2 head-group
(8 heads each).  Each core computes its batch element's attention for its 8
heads plus the partial c_proj (W_proj column-shard); the host sums the two
head-group partials per batch element.

Per-core pipeline (all layouts chosen so no on-chip input transpose is ever
needed; the host feeds pre-transposed xT / W tiles):
  stage 1 (fp32r): qkT[j,t] = WqkT^T-contract(xT)   j packs head pairs as
                   [Qa|Qb] / [Ka|Kb] 128-row chunks so stage 2 can row-tile.
                   V[t,jv]  = xT-contract(WvT), stored bf16 with a ones
                   column appended per head ([V_h | 1], 65 cols).
  stage 2 (fp32r): S.T[s,tq] = Ka/Kb lhsT vs Qa/Qb rhs, two heads run
                   concurrently on the PE via tile_position (0,0)/(64,0).
  exp (ACT):       P = exp(S.T/8) -> bf16; causal mask-mul on the 4 boundary
                   s-tiles per 512-wide tq block (host-fed 0/1 masks).
  stage 3 (bf16):  O[tq, 65] = P^T-contract([V|1]); col 64 = softmax denom.
                   Normalize with reciprocal + per-partition scalar mul while
                   copying PSUM->SBUF.
  transpose (PE):  y[t,j] -> yT[j,t] in 128x128 blocks (fp32 + identity).
  stage 4 (fp32r): out[t,co] = yT lhsT vs WpT rhs, accumulate over j.

`reps` > 1 repeats the whole body inside one NEFF (for wall-clock timing by
differencing, since per-dispatch overhead through axon is ~70 ms).
"""
import numpy as np
import ml_dtypes

import concourse.bacc as bacc
import concourse.mybir as mybir
import concourse.tile as tile
from concourse.bass_utils import run_bass_kernel_spmd

F32 = mybir.dt.float32
F32R = mybir.dt.float32r
BF16 = mybir.dt.bfloat16

B, C, NH, HD = 4, 1024, 16, 64
HPC = 8              # heads per core
JV = HPC * HD        # 512: v-feature cols per core
KC = C // 128        # 8 contraction chunks
SCALE = 1.0 / 8.0    # 1/sqrt(HD)

# bisect flags (sim-vs-HW divergence hunting)
EXP_MERGE = True      # one [128,1024] exp per pair vs two [128,512]
EXP_ABLATE = False    # timing probe: exp only [128,128] (wrong numerics)
MASK_GPSIMD = True    # causal mask-mul on POOL vs DVE
INTERLEAVE = False    # fuse stage-1 per t-block with attention (v3)


def emit_body_il(nc, tc, dram, T):
    """v3: stage-1 streamed per 512-token block, attention(tqb) emitted right
    after its block so ACT/PE overlap stage 1; all pools coexist (no phase
    barrier)."""
    TT = T // 128
    TQB = T // 512
    xT, wqkT, wvT, wpT, masks, iden, yout = (
        dram["xT"], dram["wqkT"], dram["wvT"], dram["wpT"],
        dram["masks"], dram["iden"], dram["yout"])

    with tc.tile_pool(name="persist", bufs=1) as pers, \
         tc.tile_pool(name="weights", bufs=1) as wpool, \
         tc.tile_pool(name="xstream", bufs=1) as xpool, \
         tc.tile_pool(name="pexp", bufs=1) as ppool, \
         tc.tile_pool(name="ypool", bufs=1) as ypool, \
         tc.tile_pool(name="ytpool", bufs=1) as ytpool, \
         tc.tile_pool(name="rcpool", bufs=8) as rcpool, \
         tc.tile_pool(name="outp", bufs=2) as outp, \
         tc.tile_pool(name="s2ps", bufs=2, space="PSUM") as s2ps, \
         tc.tile_pool(name="mmx", bufs=2, space="PSUM") as ps512, \
         tc.tile_pool(name="ps3p", bufs=2, space="PSUM") as ps3p:
        qkT_sb = pers.tile([128, 8, T], F32R)
        vext_sb = pers.tile([128, TT, HPC, 65], BF16)
        masks_sb = pers.tile([128, 4, 512], BF16)
        iden_bf = pers.tile([128, 128], BF16)
        nc.sync.dma_start(iden_bf[:], dram["idenb"][:])

        wqk_sb = wpool.tile([128, KC, 1024], F32R)
        nc.sync.dma_start(
            masks_sb[:], masks.rearrange("q p f -> p q f")[:, :, 0:512])
        wv_sb = wpool.tile([128, KC, JV], F32R)
        wp_sb = wpool.tile([128, 4, C], BF16)
        wqk3 = wqkT.rearrange("(kc p) j -> p kc j", p=128)
        wv3 = wvT.rearrange("(kc p) j -> p kc j", p=128)
        wp3 = dram["wpTb"].rearrange("(jc p) co -> p jc co", p=128)
        for kc in range(KC):
            nc.sync.dma_start(wqk_sb[:, kc, :], wqk3[:, kc, :])
            nc.sync.dma_start(wv_sb[:, kc, :], wv3[:, kc, :])
        for jc in range(4):
            nc.sync.dma_start(wp_sb[:, jc, :], wp3[:, jc, :])

        xT3 = xT.rearrange("(kc p) t -> p kc t", p=128)
        for tqb in range(TQB):
            nbs = slice(tqb * 512, (tqb + 1) * 512)
            # --- stage 1 for this 512-token block ---
            xT_nb = xpool.tile([128, KC, 512], F32R, tag="x")
            for kc in range(KC):
                nc.sync.dma_start(xT_nb[:, kc, :], xT3[:, kc, nbs])
            for jc in range(8):
                ps = ps512.tile([128, 512], F32, tag="ps512")
                for kc in range(KC):
                    nc.tensor.matmul(
                        ps[:], wqk_sb[:, kc, jc * 128:(jc + 1) * 128],
                        xT_nb[:, kc, :],
                        start=(kc == 0), stop=(kc == KC - 1))
                nc.vector.tensor_copy(qkT_sb[:, jc, nbs], ps[:])
            for sub in range(4):
                tt = tqb * 4 + sub
                ps = ps512.tile([128, 512], F32, tag="ps512")
                for kc in range(KC):
                    nc.tensor.matmul(
                        ps[:], xT_nb[:, kc, sub * 128:(sub + 1) * 128],
                        wv_sb[:, kc, :],
                        start=(kc == 0), stop=(kc == KC - 1))
                nc.vector.tensor_copy(
                    vext_sb[:, tt, :, 0:64],
                    ps[:].rearrange("p (h d) -> p h d", h=HPC))
                nc.vector.memset(vext_sb[:, tt, :, 64:65], 1.0)

            # --- attention for this tq block ---
            nst = 4 * (tqb + 1)
            y_t = ypool.tile([128, 4, 512], BF16, tag="y")
            for pc in range(4):
                pab = ppool.tile([128, TT, 1024], BF16, tag="pab")
                qs, ks = 2 * pc, 2 * pc + 1
                tqs = nbs
                for st in range(nst):
                    ss = slice(st * 128, (st + 1) * 128)
                    psAB = s2ps.tile([128, 1024], F32, tag="s2")
                    nc.tensor.matmul(
                        psAB[:, 0:512], qkT_sb[0:64, ks, ss],
                        qkT_sb[0:64, qs, tqs],
                        start=True, stop=True, tile_position=(0, 0))
                    nc.tensor.matmul(
                        psAB[:, 512:1024], qkT_sb[64:128, ks, ss],
                        qkT_sb[64:128, qs, tqs],
                        start=True, stop=True, tile_position=(64, 0))
                    nc.scalar.activation(
                        pab[:, st, :], psAB[:],
                        mybir.ActivationFunctionType.Exp, scale=SCALE)
                    q = st - 4 * tqb
                    if q >= 0:
                        eng = nc.gpsimd if MASK_GPSIMD else nc.vector
                        eng.tensor_mul(
                            pab[:, st, 0:512], pab[:, st, 0:512],
                            masks_sb[:, q, :])
                        eng.tensor_mul(
                            pab[:, st, 512:1024], pab[:, st, 512:1024],
                            masks_sb[:, q, :])
                for hoff in (0, 1):
                    h = 2 * pc + hoff
                    for sub in range(4):
                        ps3 = ps3p.tile([128, 65], F32, tag="s3")
                        for st in range(nst):
                            nc.tensor.matmul(
                                ps3[:],
                                pab[:, st, hoff * 512 + sub * 128:
                                    hoff * 512 + (sub + 1) * 128],
                                vext_sb[:, st, h, :],
                                start=(st == 0), stop=(st == nst - 1))
                        rc = rcpool.tile([128, 1], F32, tag="rc")
                        nc.vector.reciprocal(rc[:], ps3[:, 64:65])
                        nc.vector.tensor_scalar_mul(
                            y_t[:, sub, h * 64:(h + 1) * 64],
                            ps3[:, 0:64], rc[:])
            yT_t = ytpool.tile([128, 4, 512], BF16, tag="yt")
            for sub in range(4):
                for jc in range(4):
                    pst = ps512.tile([128, 1024], BF16, tag="ps512")
                    nc.tensor.transpose(
                        pst[:, 0:128], y_t[:, sub, jc * 128:(jc + 1) * 128],
                        iden_bf[:])
                    nc.vector.tensor_copy(
                        yT_t[:, jc, sub * 128:(sub + 1) * 128], pst[:, 0:128])
            for sub in range(4):
                for nb2 in range(2):
                    ps4 = ps512.tile([128, 512], F32, tag="ps512")
                    for jc in range(4):
                        nc.tensor.matmul(
                            ps4[:],
                            yT_t[:, jc, sub * 128:(sub + 1) * 128],
                            wp_sb[:, jc, nb2 * 512:(nb2 + 1) * 512],
                            start=(jc == 0), stop=(jc == 3))
                    ot = outp.tile([128, 512], F32, tag="ot")
                    nc.vector.tensor_copy(ot[:], ps4[:])
                    t0 = (tqb * 4 + sub) * 128
                    nc.sync.dma_start(
                        yout[t0:t0 + 128, nb2 * 512:(nb2 + 1) * 512], ot[:])


def emit_body(nc, tc, dram, T):
    TT = T // 128
    TQB = T // 512
    xT, wqkT, wvT, wpT, masks, iden, yout = (
        dram["xT"], dram["wqkT"], dram["wvT"], dram["wpT"],
        dram["masks"], dram["iden"], dram["yout"])

    with tc.tile_pool(name="persist", bufs=1) as pers:
        qkT_sb = pers.tile([128, 8, T], F32R)          # [j-part, jc, t]
        vext_sb = pers.tile([128, TT, HPC, 65], BF16)  # [s-part, st, h, d|1]
        masks_sb = pers.tile([128, 4, 1024], BF16)
        iden_sb = pers.tile([128, 128], F32)
        nc.sync.dma_start(masks_sb[:], masks.rearrange("q p f -> p q f"))
        nc.sync.dma_start(iden_sb[:], iden[:])

        with tc.tile_pool(name="s2ps", bufs=2, space="PSUM") as s2ps, \
             tc.tile_pool(name="mmx", bufs=2, space="PSUM") as ps512, \
             tc.tile_pool(name="ps3p", bufs=2, space="PSUM") as ps3p:
            pstp = ps512

            # ---------------- stage 1 ----------------
            with tc.tile_pool(name="stage1", bufs=1) as s1p:
                xT_sb = s1p.tile([128, KC, T], F32R)
                wqk_sb = s1p.tile([128, KC, 1024], F32R)
                wv_sb = s1p.tile([128, KC, JV], F32R)
                xT3 = xT.rearrange("(kc p) t -> p kc t", p=128)
                wqk3 = wqkT.rearrange("(kc p) j -> p kc j", p=128)
                wv3 = wvT.rearrange("(kc p) j -> p kc j", p=128)
                for kc in range(KC):
                    nc.sync.dma_start(xT_sb[:, kc, :], xT3[:, kc, :])
                    nc.sync.dma_start(wqk_sb[:, kc, :], wqk3[:, kc, :])
                    nc.sync.dma_start(wv_sb[:, kc, :], wv3[:, kc, :])

                # qkT = WqkT.T-contract(xT): out chunk jc over t blocks
                for jc in range(8):
                    for nb in range(TQB):
                        ps = ps512.tile([128, 512], F32, tag="ps512")
                        for kc in range(KC):
                            nc.tensor.matmul(
                                ps[:],
                                wqk_sb[:, kc, jc * 128:(jc + 1) * 128],
                                xT_sb[:, kc, nb * 512:(nb + 1) * 512],
                                start=(kc == 0), stop=(kc == KC - 1))
                        nc.vector.tensor_copy(
                            qkT_sb[:, jc, nb * 512:(nb + 1) * 512], ps[:])
                # V = xT.T-contract(WvT): out t-chunk tt, 512 v-cols
                for tt in range(TT):
                    ps = ps512.tile([128, 512], F32, tag="ps512")
                    for kc in range(KC):
                        nc.tensor.matmul(
                            ps[:],
                            xT_sb[:, kc, tt * 128:(tt + 1) * 128],
                            wv_sb[:, kc, :],
                            start=(kc == 0), stop=(kc == KC - 1))
                    nc.vector.tensor_copy(
                        vext_sb[:, tt, :, 0:64],
                        ps[:].rearrange("p (h d) -> p h d", h=HPC))
                    nc.vector.memset(vext_sb[:, tt, :, 64:65], 1.0)

            # ---------------- attention + proj ----------------
            with tc.tile_pool(name="wp", bufs=1) as wpp, \
                 tc.tile_pool(name="pexp", bufs=2) as ppool, \
                 tc.tile_pool(name="ypool", bufs=2) as ypool, \
                 tc.tile_pool(name="ytpool", bufs=2) as ytpool, \
                 tc.tile_pool(name="rcpool", bufs=8) as rcpool, \
                 tc.tile_pool(name="outp", bufs=3) as outp:
                wp_sb = wpp.tile([128, 4, C], F32R)
                wp3 = wpT.rearrange("(jc p) co -> p jc co", p=128)
                for jc in range(4):
                    nc.sync.dma_start(wp_sb[:, jc, :], wp3[:, jc, :])

                for tqb in range(TQB):
                    nst = 4 * (tqb + 1)     # causal: s-tiles 0..nst-1
                    y_t = ypool.tile([128, 4, 512], F32, tag="y")
                    for pc in range(4):
                        pab = ppool.tile([128, TT, 1024], BF16, tag="pab")
                        qs = 2 * pc         # chunk with [Qa|Qb]
                        ks = 2 * pc + 1     # chunk with [Ka|Kb]
                        tqs = slice(tqb * 512, (tqb + 1) * 512)
                        for st in range(nst):
                            ss = slice(st * 128, (st + 1) * 128)
                            psAB = s2ps.tile([128, 1024], F32, tag="s2")
                            nc.tensor.matmul(
                                psAB[:, 0:512], qkT_sb[0:64, ks, ss],
                                qkT_sb[0:64, qs, tqs],
                                start=True, stop=True, tile_position=(0, 0))
                            nc.tensor.matmul(
                                psAB[:, 512:1024], qkT_sb[64:128, ks, ss],
                                qkT_sb[64:128, qs, tqs],
                                start=True, stop=True, tile_position=(64, 0))
                            if EXP_ABLATE:
                                nc.scalar.activation(
                                    pab[:, st, 0:128], psAB[:, 0:128],
                                    mybir.ActivationFunctionType.Exp, scale=SCALE)
                            elif EXP_MERGE:
                                nc.scalar.activation(
                                    pab[:, st, :], psAB[:],
                                    mybir.ActivationFunctionType.Exp, scale=SCALE)
                            else:
                                nc.scalar.activation(
                                    pab[:, st, 0:512], psAB[:, 0:512],
                                    mybir.ActivationFunctionType.Exp, scale=SCALE)
                                nc.scalar.activation(
                                    pab[:, st, 512:1024], psAB[:, 512:1024],
                                    mybir.ActivationFunctionType.Exp, scale=SCALE)
                            q = st - 4 * tqb
                            if q >= 0:      # boundary tile: causal mask
                                eng = nc.gpsimd if MASK_GPSIMD else nc.vector
                                eng.tensor_mul(
                                    pab[:, st, :], pab[:, st, :], masks_sb[:, q, :])
                        for hoff in (0, 1):
                            h = 2 * pc + hoff
                            for sub in range(4):
                                ps3 = ps3p.tile([128, 65], F32, tag="s3")
                                for st in range(nst):
                                    nc.tensor.matmul(
                                        ps3[:],
                                        pab[:, st,
                                            hoff * 512 + sub * 128:
                                            hoff * 512 + (sub + 1) * 128],
                                        vext_sb[:, st, h, :],
                                        start=(st == 0), stop=(st == nst - 1))
                                rc = rcpool.tile([128, 1], F32, tag="rc")
                                nc.vector.reciprocal(rc[:], ps3[:, 64:65])
                                nc.vector.tensor_scalar_mul(
                                    y_t[:, sub, h * 64:(h + 1) * 64],
                                    ps3[:, 0:64], rc[:])
                    # transpose y [t, j] -> yT [j, t] for this tq block
                    yT_t = ytpool.tile([128, 4, 512], F32R, tag="yt")
                    for sub in range(4):
                        for jc in range(4):
                            pst = pstp.tile([128, 512], F32, tag="ps512")
                            nc.tensor.transpose(
                                pst[:, 0:128], y_t[:, sub, jc * 128:(jc + 1) * 128],
                                iden_sb[:])
                            nc.vector.tensor_copy(
                                yT_t[:, jc, sub * 128:(sub + 1) * 128], pst[:, 0:128])
                    # stage 4: out[t, co] partial for this tq block
                    for sub in range(4):
                        for nb2 in range(2):
                            ps4 = ps512.tile([128, 512], F32, tag="ps512")
                            for jc in range(4):
                                nc.tensor.matmul(
                                    ps4[:],
                                    yT_t[:, jc, sub * 128:(sub + 1) * 128],
                                    wp_sb[:, jc, nb2 * 512:(nb2 + 1) * 512],
                                    start=(jc == 0), stop=(jc == 3))
                            ot = outp.tile([128, 512], F32, tag="ot")
                            nc.vector.tensor_copy(ot[:], ps4[:])
                            t0 = (tqb * 4 + sub) * 128
                            nc.sync.dma_start(
                                yout[t0:t0 + 128, nb2 * 512:(nb2 + 1) * 512],
                                ot[:])


def build_nc(T=2048, reps=1):
    nc = bacc.Bacc()
    dram = dict(
        xT=nc.dram_tensor("xT", [C, T], F32R, kind="ExternalInput"),
        wqkT=nc.dram_tensor("wqkT", [C, 1024], F32R, kind="ExternalInput"),
        wvT=nc.dram_tensor("wvT", [C, JV], F32R, kind="ExternalInput"),
        wpT=nc.dram_tensor("wpT", [JV, C], F32R, kind="ExternalInput"),
        wpTb=nc.dram_tensor("wpTb", [JV, C], BF16, kind="ExternalInput"),
        masks=nc.dram_tensor("masks", [4, 128, 1024], BF16, kind="ExternalInput"),
        iden=nc.dram_tensor("iden", [128, 128], F32, kind="ExternalInput"),
        idenb=nc.dram_tensor("idenb", [128, 128], BF16, kind="ExternalInput"),
        yout=nc.dram_tensor("yout", [T, C], F32, kind="ExternalOutput"),
    )
    with tile.TileContext(nc) as tc:
        for _ in range(reps):
            (emit_body_il if INTERLEAVE else emit_body)(nc, tc, dram, T)
    nc.compile()
    return nc


def shard_inputs(x, W_attn, W_proj, T):
    """Full inputs -> list of 8 per-core in_maps."""
    x = np.asarray(x, dtype=np.float32)
    W_attn = np.asarray(W_attn, dtype=np.float32)
    W_proj = np.asarray(W_proj, dtype=np.float32)

    sp = np.arange(128)[:, None]
    tf = np.arange(512)[None, :]
    m1 = np.stack([(tf >= sp + q * 128) for q in range(4)])
    masks = np.concatenate([m1, m1], axis=2).astype(ml_dtypes.bfloat16)
    iden = np.eye(128, dtype=np.float32)

    in_maps = []
    for core in range(8):
        b, hg = core // 2, core % 2
        heads = [hg * HPC + i for i in range(HPC)]
        cols = []
        for pc in range(4):
            ha, hb = heads[2 * pc], heads[2 * pc + 1]
            cols += list(range(ha * 192, ha * 192 + 64))        # Q_a
            cols += list(range(hb * 192, hb * 192 + 64))        # Q_b
            cols += list(range(ha * 192 + 64, ha * 192 + 128))  # K_a
            cols += list(range(hb * 192 + 64, hb * 192 + 128))  # K_b
        vrows = [h * 192 + 128 + d for h in heads for d in range(64)]
        in_maps.append(dict(
            xT=np.ascontiguousarray(x[b, :T].T),
            wqkT=np.ascontiguousarray(W_attn[cols].T),
            wvT=np.ascontiguousarray(W_attn[vrows].T),
            wpT=np.ascontiguousarray(W_proj[:, hg * JV:(hg + 1) * JV].T),
            masks=masks, iden=iden,
            idenb=iden.astype(ml_dtypes.bfloat16),
            wpTb=np.ascontiguousarray(
                W_proj[:, hg * JV:(hg + 1) * JV].T).astype(ml_dtypes.bfloat16),
        ))
    return in_maps


def gather_outputs(results, T):
    out = np.empty((B, T, C), dtype=np.float32)
    for b in range(B):
        out[b] = results[2 * b]["yout"] + results[2 * b + 1]["yout"]
    return out


_NC_CACHE = {}


def run(x, W_attn, W_proj, T=2048, trace=False):
    if T not in _NC_CACHE:
        _NC_CACHE[T] = build_nc(T)
    nc = _NC_CACHE[T]
    in_maps = shard_inputs(x, W_attn, W_proj, T)
    res = run_bass_kernel_spmd(nc, in_maps, core_ids=list(range(8)), trace=trace)
    return gather_outputs(res.results, T), res


def kernel(x, W_attn, W_proj):
    out, _ = run(x, W_attn, W_proj, T=2048)
    return out
